# revision 1
# baseline (speedup 1.0000x reference)
"""Causal self-attention Trainium2 kernel (8 NeuronCores, SPMD).

Problem (hardcoded): x [4, 2048, 2048] f32, W_qkv [6144, 2048], W_out [2048, 2048],
16 heads x 128 dim, causal softmax attention + output projection.

Sharding: core c = 2*b + g handles batch b (4) and head-group g (2 groups of 8
heads).  Each core computes its 8 heads' QKV projection, attention, and the
partial output projection against its slice of W_out columns; the host sums the
two partials per batch element.

All matmuls run as float32r (full-rate fp32 on the PE array).  Per-head
attention computes S^T = K.Q^T tiles (k on partitions, q on the free dim) so
softmax renormalization sums arrive via a ones-vector matmul and the AV matmul
(lhsT = V) directly produces O^T, which feeds the output projection as lhsT.
Causal masking skips strictly-upper k-blocks and masks diagonal blocks.
"""

import math

import numpy as np

B = 4
T = 2048
C = 2048
H = 16          # total heads
HG = 8          # heads per core (tensor-parallel group)
D = 128         # head dim
P = 128         # partitions
NCS = C // P    # 16 contraction subtiles
NTC = T // P    # 16 T chunks of 128
NTB = T // 512  # 4 T blocks of 512
SCALE = 1.0 / math.sqrt(D)

_CACHED = None


def _build(phases="abc", repeat=1, vmode="vt"):
    import concourse.mybir as mybir
    from concourse import bacc
    from concourse.tile import TileContext

    f32 = mybir.dt.float32
    f32r = mybir.dt.float32r
    EXP = mybir.ActivationFunctionType.Exp
    MULT = mybir.AluOpType.mult

    nc = bacc.Bacc("TRN2", target_bir_lowering=False)

    xt_d = nc.dram_tensor("xt", [NCS, P, T], f32r, kind="ExternalInput")
    wq_d = nc.dram_tensor("wq", [HG, P, NCS, D], f32r, kind="ExternalInput")
    wk_d = nc.dram_tensor("wk", [HG, P, NCS, D], f32r, kind="ExternalInput")
    if vmode == "vt":
        wv_d = nc.dram_tensor("wv", [HG, P, NCS, D], f32r, kind="ExternalInput")
        ident_d = nc.dram_tensor("ident", [P, P], f32r, kind="ExternalInput")
    else:
        wv_d = nc.dram_tensor("wv", [P, NCS, HG * D], f32r, kind="ExternalInput")
    wo_d = nc.dram_tensor("wo", [HG * D, C], f32r, kind="ExternalInput")
    masks_d = nc.dram_tensor("masks", [4, P, 512], f32r, kind="ExternalInput")
    ones_d = nc.dram_tensor("ones", [P, 1], f32r, kind="ExternalInput")
    onesrow_d = nc.dram_tensor("ones_row", [1, P], f32r, kind="ExternalInput")
    out_d = nc.dram_tensor("out", [T, C], f32, kind="ExternalOutput")

    with TileContext(nc) as tc:
        with tc.tile_pool(name="persist", bufs=1) as persist, \
             tc.tile_pool(name="dram", bufs=1, space="DRAM") as dram:
            masks_t = persist.tile([P, 4, 512], f32r, tag="masks")
            nc.sync.dma_start(masks_t, masks_d.rearrange("m p q -> p m q"))
            ones_t = persist.tile([P, 1], f32r, tag="ones")
            nc.sync.dma_start(ones_t, ones_d[:])
            onesrow_t = persist.tile([1, P], f32r, tag="onesrow")
            nc.sync.dma_start(onesrow_t, onesrow_d[:])
            ident_t = None
            if vmode == "vt":
                ident_t = persist.tile([P, P], f32r, tag="ident")
                nc.sync.dma_start(ident_t, ident_d[:])

            qt_dram = dram.tile([HG, P, T], f32r)    # Q^T per head
            kt_dram = dram.tile([HG, P, T], f32r)    # K^T per head
            if vmode == "vt":
                v_dram_h = []   # V row-chunks, per head
                for hh in range(HG):
                    vh_ = dram.tile([NTC, P, D], f32r, tag=f"vdramh{hh}")
                    v_dram_h.append(vh_)
            else:
                v_dram_q = []   # V row-chunks, split by column quarter
                for q in range(4):
                    vq = dram.tile([NTC, P, 256], f32r, tag=f"vdram{q}")
                    v_dram_q.append(vq)

            for _rep in range(repeat):
                # ---------------- Phase A: QKV projections ----------------
                with tc.tile_pool(name="xt", bufs=1) as xtp, \
                     tc.tile_pool(name="aw", bufs=2) as awp, \
                     tc.tile_pool(name="awv", bufs=2) as awvp, \
                     tc.tile_pool(name="astage", bufs=6) as astage, \
                     tc.tile_pool(name="apsum", bufs=4, space="PSUM") as apsum:
                    # first weight tile before the xt bulk so PE can start
                    wt0 = awp.tile([P, NCS, D], f32r, tag="aw")
                    nc.sync.dma_start(wt0, wq_d[0])
                    xt = []
                    for cs in range(NCS):
                        t_ = xtp.tile([P, T], f32r, tag=f"xt{cs}")
                        nc.sync.dma_start(t_, xt_d[cs])
                        xt.append(t_)
                    # prefetch first two V-weight quarters (direct mode)
                    wvq_pre = []
                    if vmode != "vt":
                        for q in range(2):
                            wvq = awvp.tile([P, NCS, 256], f32r, tag="awv")
                            nc.sync.dma_start(
                                wvq, wv_d[:, :, q * 256:(q + 1) * 256])
                            wvq_pre.append(wvq)

                    # Q^T and K^T: per head h, psum[128, 512] over T-blocks,
                    # accumulating over the 16 C-subtiles.
                    for w_d, dst in ((wq_d, qt_dram), (wk_d, kt_dram)):
                        for h in range(HG):
                            if w_d is wq_d and h == 0:
                                wt = wt0
                            else:
                                wt = awp.tile([P, NCS, D], f32r, tag="aw")
                                nc.sync.dma_start(wt, w_d[h])
                            for tb in range(NTB):
                                ps = apsum.tile([P, 512], f32, tag="aps")
                                for cs in range(NCS):
                                    nc.tensor.matmul(
                                        ps, wt[:, cs], xt[cs][:, tb * 512:(tb + 1) * 512],
                                        start=(cs == 0), stop=(cs == NCS - 1))
                                st = astage.tile([P, 512], f32r, tag="ast")
                                nc.vector.tensor_copy(out=st, in_=ps)
                                nc.sync.dma_start(dst[h][:, tb * 512:(tb + 1) * 512], st)

                    if vmode == "vt":
                        # V^T per head like Q/K (N=512, LDW hidden), then
                        # PE-transpose 128x128 blocks into V row layout.
                        with tc.tile_pool(name="atp", bufs=2,
                                          space="PSUM") as atpp:
                            for h in range(HG):
                                wvh = awp.tile([P, NCS, D], f32r, tag="aw")
                                nc.sync.dma_start(wvh, wv_d[h])
                                for tb in range(NTB):
                                    ps = apsum.tile([P, 512], f32, tag="aps")
                                    for cs in range(NCS):
                                        nc.tensor.matmul(
                                            ps, wvh[:, cs],
                                            xt[cs][:, tb * 512:(tb + 1) * 512],
                                            start=(cs == 0),
                                            stop=(cs == NCS - 1))
                                    st = astage.tile([P, 512], f32r, tag="ast")
                                    nc.vector.tensor_copy(out=st, in_=ps)
                                    for j in range(4):
                                        tch = tb * 4 + j
                                        tp_ = atpp.tile([P, P], f32r, tag="atp")
                                        nc.tensor.transpose(
                                            tp_, st[:, j * P:(j + 1) * P],
                                            ident_t)
                                        vst = astage.tile([P, P], f32r,
                                                          tag="avs")
                                        nc.vector.tensor_copy(out=vst, in_=tp_)
                                        nc.sync.dma_start(
                                            v_dram_h[h][tch], vst)
                    else:
                        # V in [T, heads*D] layout directly: lhsT = xT chunk.
                        for q in range(4):
                            if q < 2:
                                wvq = wvq_pre[q]
                            else:
                                wvq = awvp.tile([P, NCS, 256], f32r, tag="awv")
                                nc.sync.dma_start(
                                    wvq, wv_d[:, :, q * 256:(q + 1) * 256])
                            for tch in range(NTC):
                                ps = apsum.tile([P, 256], f32, tag="apv")
                                for cs in range(NCS):
                                    nc.tensor.matmul(
                                        ps, xt[cs][:, tch * P:(tch + 1) * P],
                                        wvq[:, cs],
                                        start=(cs == 0), stop=(cs == NCS - 1))
                                st = astage.tile([P, 256], f32r, tag="asv")
                                nc.vector.tensor_copy(out=st, in_=ps)
                                nc.sync.dma_start(v_dram_q[q][tch], st)

                # ---------------- Phase B: attention per head --------------
                with tc.tile_pool(name="ot", bufs=1) as otp, \
                     tc.tile_pool(name="cw", bufs=2) as cwp:
                    ot = []
                    wo_r = wo_d.rearrange("(h p) o -> p h o", p=P)
                    wo_pre = None
                    if "c" in phases:
                        wo_pre = cwp.tile([P, HG, 512], f32r, tag="cw")
                        nc.sync.dma_start(wo_pre, wo_r[:, :, 0:512])
                    with tc.tile_pool(name="bhead", bufs=2) as bh, \
                         tc.tile_pool(name="bpt", bufs=6) as bptp, \
                         tc.tile_pool(name="bmisc", bufs=4) as bmisc, \
                         tc.tile_pool(name="bpsum", bufs=2, space="PSUM") as bps, \
                         tc.tile_pool(name="bpsum3", bufs=4, space="PSUM") as bps3, \
                         tc.tile_pool(name="bpsum1", bufs=1, space="PSUM") as bps1:
                        def load_head(h):
                            qt_t = bh.tile([P, T], f32r, tag="qt")
                            nc.sync.dma_start(qt_t, qt_dram[h])
                            kt_t = bh.tile([P, T], f32r, tag="kt")
                            nc.sync.dma_start(kt_t, kt_dram[h])
                            v_t = bh.tile([P, NTC, D], f32r, tag="vh")
                            if vmode == "vt":
                                nc.sync.dma_start(
                                    v_t, v_dram_h[h].rearrange(
                                        "tc p d -> p tc d"))
                            else:
                                hd = (h % 2) * D
                                nc.sync.dma_start(
                                    v_t, v_dram_q[h // 2].rearrange(
                                        "tc p hd -> p tc hd")[:, :, hd:hd + D])
                            return qt_t, kt_t, v_t
                        head0 = load_head(0) if "b" in phases else None
                        for h in range(HG if "b" in phases else 0):
                            if h == 0:
                                qt_t, kt_t, v_t = head0
                            else:
                                qt_t, kt_t, v_t = load_head(h)

                            ot_h = otp.tile([P, T], f32r, tag=f"ot{h}")
                            ot.append(ot_h)

                            for jb in range(NTB):   # q-blocks of 512
                                nk = 4 * (jb + 1)   # causal: k-chunks 0..nk-1
                                po = bps.tile([P, 512], f32, tag="po")
                                psum_s = bps1.tile([P, 512], f32, tag="psu")
                                # software-pipelined: S-matmul for ks+1 issues
                                # before the AV/sum matmuls of ks, so PE never
                                # waits on the exp of the current tile.
                                pts = {}
                                def s_step(ks):
                                    # columns q < m*128 of a diagonal
                                    # sub-chunk are fully masked: shrink the
                                    # moving width (only while >= 256 so
                                    # f32r stays full-rate)
                                    m = ks - 4 * jb
                                    q0 = m * P if m in (1, 2) else 0
                                    pst = bps3.tile([P, 512], f32, tag="pst")
                                    nc.tensor.matmul(
                                        pst[:, q0:],
                                        kt_t[:, ks * P:(ks + 1) * P],
                                        qt_t[:, jb * 512 + q0:(jb + 1) * 512],
                                        start=True, stop=True)
                                    pt = bptp.tile([P, 512], f32r, tag="pt")
                                    nc.scalar.activation(
                                        pt[:, q0:], pst[:, q0:], EXP,
                                        scale=SCALE)
                                    if m >= 0:  # diagonal block: causal mask
                                        nc.vector.tensor_tensor(
                                            pt[:, q0:], pt[:, q0:],
                                            masks_t[:, m, q0:], MULT)
                                    pts[ks] = (pt, q0)
                                s_step(0)
                                for ks in range(nk):
                                    if ks + 1 < nk:
                                        s_step(ks + 1)
                                    pt, q0 = pts.pop(ks)
                                    nc.tensor.matmul(
                                        po[:, q0:], v_t[:, ks], pt[:, q0:],
                                        start=(ks == 0), stop=(ks == nk - 1))
                                    nc.tensor.matmul(
                                        psum_s[0:1, q0:], ones_t, pt[:, q0:],
                                        start=(ks == 0), stop=(ks == nk - 1))
                                # normalize: recip of rowsums, bcast via PE
                                recip = bmisc.tile([1, 512], f32r, tag="rc")
                                with nc.allow_low_precision("f32r recip ok"):
                                    nc.vector.reciprocal(recip, psum_s[0:1])
                                pb = bps1.tile([P, 512], f32, tag="pb")
                                nc.tensor.matmul(pb, onesrow_t, recip,
                                                 start=True, stop=True)
                                cpb = bmisc.tile([P, 512], f32, tag="cpb")
                                nc.vector.tensor_copy(out=cpb, in_=pb)
                                nc.vector.tensor_tensor(
                                    ot_h[:, jb * 512:(jb + 1) * 512], po, cpb, MULT)

                    # ------------- Phase C: output projection --------------
                    if "c" not in phases:
                        with tc.tile_pool(name="dummy", bufs=1) as dp:
                            z = dp.tile([P, 512], f32, tag="z")
                            nc.vector.memset(z, 0.0)
                            for tch in range(NTC):
                                for ob in range(4):
                                    nc.sync.dma_start(
                                        out_d[tch * P:(tch + 1) * P,
                                              ob * 512:(ob + 1) * 512], z)
                        phases_c = False
                    else:
                        phases_c = True
                    with tc.tile_pool(name="cstage", bufs=6) as cstage, \
                         tc.tile_pool(name="cpsum", bufs=4, space="PSUM") as cps:
                        for ob in range(4 if phases_c else 0):     # output blocks of 512
                            if ob == 0:
                                wo_t = wo_pre
                            else:
                                wo_t = cwp.tile([P, HG, 512], f32r, tag="cw")
                                nc.sync.dma_start(
                                    wo_t, wo_r[:, :, ob * 512:(ob + 1) * 512])
                            for tch in range(NTC):
                                ps = cps.tile([P, 512], f32, tag="cps")
                                for h in range(HG):
                                    nc.tensor.matmul(
                                        ps, ot[h][:, tch * P:(tch + 1) * P],
                                        wo_t[:, h],
                                        start=(h == 0), stop=(h == HG - 1))
                                st = cstage.tile([P, 512], f32, tag="cst")
                                nc.vector.tensor_copy(out=st, in_=ps)
                                nc.sync.dma_start(
                                    out_d[tch * P:(tch + 1) * P,
                                          ob * 512:(ob + 1) * 512], st)

    nc.finalize()
    return nc


VMODE = "direct"


def _get_nc():
    global _CACHED
    if _CACHED is None:
        _CACHED = _build(vmode=VMODE)
    return _CACHED


def _prep_inputs(x, W_qkv, W_out, vmode=None):
    """Host-side shard + layout prep. Returns per-core input maps."""
    f32 = np.float32
    x = np.asarray(x, dtype=f32)
    W_qkv = np.asarray(W_qkv, dtype=f32)
    W_out = np.asarray(W_out, dtype=f32)

    # causal masks for the 4 diagonal sub-positions of a 512-wide q block
    k_idx = np.arange(P)
    q_idx = np.arange(512)
    masks = np.stack([
        (q_idx[None, :] >= (m * P + k_idx)[:, None]).astype(f32)
        for m in range(4)
    ])  # [4, 128, 512]
    ones = np.ones((P, 1), dtype=f32)
    ones_row = np.ones((1, P), dtype=f32)
    if vmode is None:
        vmode = VMODE
    ident = np.eye(P, dtype=f32)

    per_g = {}
    for g in range(2):
        sl = slice(g * HG * D, (g + 1) * HG * D)
        wq = W_qkv[0 * C:1 * C][sl]        # [1024, 2048]
        wk = W_qkv[1 * C:2 * C][sl]
        wv = W_qkv[2 * C:3 * C][sl]
        # [h, p, cs, m]: element = w[h*128+m, cs*128+p]
        wq_a = np.ascontiguousarray(
            wq.reshape(HG, D, NCS, P).transpose(0, 3, 2, 1))
        wk_a = np.ascontiguousarray(
            wk.reshape(HG, D, NCS, P).transpose(0, 3, 2, 1))
        if vmode == "vt":
            # same layout as wq/wk: [h, p, cs, d]
            wv_a = np.ascontiguousarray(
                wv.reshape(HG, D, NCS, P).transpose(0, 3, 2, 1))
        else:
            # [p, cs, hm]: element = wv[hm, cs*128+p]
            wv_a = np.ascontiguousarray(
                wv.reshape(HG * D, NCS, P).transpose(2, 1, 0))
        wo_a = np.ascontiguousarray(W_out[:, sl].T)   # [1024, 2048]
        per_g[g] = (wq_a, wk_a, wv_a, wo_a)

    in_maps = []
    for core in range(8):
        b, g = divmod(core, 2)
        xt = np.ascontiguousarray(x[b].T).reshape(NCS, P, T)
        wq_a, wk_a, wv_a, wo_a = per_g[g]
        im = {
            "xt": xt, "wq": wq_a, "wk": wk_a, "wv": wv_a, "wo": wo_a,
            "masks": masks, "ones": ones, "ones_row": ones_row,
        }
        if vmode == "vt":
            im["ident"] = ident
        in_maps.append(im)
    return in_maps


def kernel(x, W_qkv, W_out, *, trace=False, trace_cores=None):
    from concourse.bass_utils import run_bass_kernel_spmd

    nc = _get_nc()
    in_maps = _prep_inputs(x, W_qkv, W_out)
    r = run_bass_kernel_spmd(
        nc, in_maps, core_ids=list(range(8)),
        trace=trace, trace_cores=trace_cores)

    out = np.empty((B, T, C), dtype=np.float32)
    for b in range(B):
        out[b] = r.results[2 * b]["out"] + r.results[2 * b + 1]["out"]
    if trace:
        kernel.last_results = r
    return out



# revision 2
# speedup vs baseline: 1.0160x; 1.0160x over previous
"""Causal self-attention Trainium2 kernel (8 NeuronCores, SPMD) — v2.

Problem (hardcoded): x [4, 2048, 2048] f32, W_qkv [6144, 2048], W_out
[2048, 2048], 16 heads x 128 dim, causal softmax attention + output
projection.

Sharding: core c = 2*b + g handles batch b (4) and head-group g (2 groups
of 8 heads).  Host sums the two partial output projections per batch.

v2 design (PE-cycle minimization; the kernel is tensor-engine bound):
  * all matmul operands in bf16 (PSUM accumulation stays f32) — bf16 runs
    1 cycle/row at any free width, so causal masking can shrink tiles to
    128 columns (f32r needs >=256-wide tiles for full rate).
  * softmax denominators accumulated on the Vector engine (f32 running
    sum of the exp tiles) instead of per-chunk ones-matmuls on the PE;
    a single ones[128,128] matmul per q-block collapses partitions AND
    broadcasts the sums to all 128 partitions (replacing the separate
    reciprocal-broadcast matmul).
  * fused per-head pipeline: head h's attention interleaves (on the PE
    instruction stream) with head h+1's QKV projection segments, so the
    Activation engine's exp chain never stalls the PE.
  * Q^T/K^T/V stay in SBUF (no DRAM round-trip); x tiles and per-head
    weights are re-DMA'd every repeat (steady-state cost), with the x
    reload for the next repeat overlapped with the output projection.
"""

import math

import numpy as np

B = 4
T = 2048
C = 2048
H = 16          # total heads
HG = 8          # heads per core (tensor-parallel group)
D = 128         # head dim
P = 128         # partitions
NCS = C // P    # 16 contraction subtiles
NTC = T // P    # 16 T chunks of 128
NTB = T // 512  # 4 T blocks of 512
SCALE = 1.0 / math.sqrt(D)

_CACHED = None
VMODE = "v2"


def _build(phases="abc", repeat=1, vmode=None):
    import concourse.mybir as mybir
    from concourse import bacc
    from concourse.tile import TileContext

    f32 = mybir.dt.float32
    f32r = mybir.dt.float32r
    bf = mybir.dt.bfloat16
    EXP = mybir.ActivationFunctionType.Exp
    MULT = mybir.AluOpType.mult
    ADD = mybir.AluOpType.add

    nc = bacc.Bacc("TRN2", target_bir_lowering=False)

    xt_d = nc.dram_tensor("xt", [NCS, P, T], bf, kind="ExternalInput")
    # packed per-head q/k weights: [h, p, r, cs, d] = W_r[h*128+d, cs*128+p]
    w_d = nc.dram_tensor("w", [HG, P, 2, NCS, D], bf, kind="ExternalInput")
    # v weights per head-PAIR (256-wide moving dim keeps the PE weight
    # loads hidden): [hp, p, cs, j] = W_v[hp*256+j, cs*128+p]
    wv2_d = nc.dram_tensor("wv2", [HG // 2, P, NCS, 2 * D], bf,
                           kind="ExternalInput")
    # output projection: [p, h, o] = W_out[o, sl + h*128 + p]
    wo_d = nc.dram_tensor("wo", [P, HG, C], bf, kind="ExternalInput")
    tri_d = nc.dram_tensor("tri", [P, P], bf, kind="ExternalInput")
    ones_d = nc.dram_tensor("ones", [P, P], f32r, kind="ExternalInput")
    out_d = nc.dram_tensor("out", [T, C], f32, kind="ExternalOutput")

    with TileContext(nc) as tc:
        with tc.tile_pool(name="persist", bufs=1) as persist, \
             tc.tile_pool(name="xt", bufs=1) as xtp, \
             tc.tile_pool(name="ot", bufs=1) as otp, \
             tc.tile_pool(name="wp", bufs=2) as wp, \
             tc.tile_pool(name="wvp", bufs=2) as wvp, \
             tc.tile_pool(name="wob", bufs=2) as wobp, \
             tc.tile_pool(name="qk", bufs=4) as qkp, \
             tc.tile_pool(name="vv", bufs=4) as vp, \
             tc.tile_pool(name="pt", bufs=5) as ptp, \
             tc.tile_pool(name="acc", bufs=2) as accp, \
             tc.tile_pool(name="rec", bufs=1) as recp, \
             tc.tile_pool(name="cst", bufs=2) as cstp, \
             tc.tile_pool(name="pa", bufs=2, space="PSUM") as pap, \
             tc.tile_pool(name="pst", bufs=3, space="PSUM") as pstp, \
             tc.tile_pool(name="po", bufs=2, space="PSUM") as pop, \
             tc.tile_pool(name="pss", bufs=1, space="PSUM") as pssp:

            tri_t = persist.tile([P, P], bf, tag="tri")
            nc.sync.dma_start(tri_t, tri_d[:])
            ones_t = persist.tile([P, P], f32r, tag="ones")
            nc.sync.dma_start(ones_t, ones_d[:])

            ot = [persist.tile([P, T], bf, tag=f"ot{h}", name=f"ot{h}")
                  for h in range(HG)]

            def dma_xt():
                xts = []
                for cs in range(NCS):
                    t_ = xtp.tile([P, T], bf, tag=f"xt{cs}")
                    nc.sync.dma_start(t_, xt_d[cs])
                    xts.append(t_)
                return xts

            def dma_w(h):
                w_t = wp.tile([P, 2, NCS, D], bf, tag="w")
                nc.sync.dma_start(w_t, w_d[h])
                if h % 2 == 0:
                    wv_t = wvp.tile([P, NCS, 2 * D], bf, tag="wv")
                    nc.sync.dma_start(wv_t, wv2_d[h // 2])
                    return (w_t, wv_t)
                return (w_t, None)

            def dma_wob(ob):
                t_ = wobp.tile([P, HG, 512], bf, tag="wob")
                nc.sync.dma_start(t_, wo_d[:, :, ob * 512:(ob + 1) * 512])
                return t_

            vtiles = {}

            def make_qkv_items(h, w_pair, xts):
                """Uniform PE segments (~8192 cycles each) computing
                Q^T, K^T (D-part x T) for head h, plus — on even h — V
                (k-part x D chunks) for the PAIR (h, h+1) with a
                256-wide moving dim so PE weight loads stay hidden.
                Returns (items, (qt, kt))."""
                w_t, wv_t = w_pair
                qt = qkp.tile([P, T], bf, tag="qt")
                kt = qkp.tile([P, T], bf, tag="kt")
                if h % 2 == 0:
                    vtiles[h] = vp.tile([P, NTC, D], bf, tag="v",
                                        name=f"v{h}")
                    vtiles[h + 1] = vp.tile([P, NTC, D], bf, tag="v",
                                            name=f"v{h + 1}")

                def qk_seg(r, tb, dst):
                    def run():
                        ps = pap.tile([P, 512], f32, tag="pa")
                        for cs in range(NCS):
                            nc.tensor.matmul(
                                ps, w_t[:, r, cs],
                                xts[cs][:, tb * 512:(tb + 1) * 512],
                                start=(cs == 0), stop=(cs == NCS - 1))
                        nc.scalar.copy(
                            out=dst[:, tb * 512:(tb + 1) * 512], in_=ps)
                    return run

                def v2_seg(jj):
                    va, vb = vtiles[h], vtiles[h + 1]

                    def run():
                        ps = pap.tile([P, 512], f32, tag="pa")
                        for i in range(2):
                            tch = jj * 2 + i
                            for cs in range(NCS):
                                nc.tensor.matmul(
                                    ps[:, i * 256:(i + 1) * 256],
                                    xts[cs][:, tch * P:(tch + 1) * P],
                                    wv_t[:, cs],
                                    start=(cs == 0), stop=(cs == NCS - 1))
                        pv = ps.rearrange("p (i h j) -> p i h j", i=2, h=2)
                        nc.scalar.copy(
                            out=va[:, jj * 2:(jj + 1) * 2, :],
                            in_=pv[:, :, 0, :])
                        nc.scalar.copy(
                            out=vb[:, jj * 2:(jj + 1) * 2, :],
                            in_=pv[:, :, 1, :])
                    return run

                items = [qk_seg(0, tb, qt) for tb in range(NTB)]
                items += [qk_seg(1, tb, kt) for tb in range(NTB)]
                if h % 2 == 0:
                    items += [v2_seg(jj) for jj in range(8)]
                return items, (qt, kt)

            def attn(h, qt, kt, fillers, pre, post):
                v = vtiles[h]
                """Attention for head h; fillers are PE work items
                (next head's QKV segs / first output-proj groups)
                interleaved so the PE never waits on the exp chain.
                pre[jb] fillers run between the last AV matmul and the
                sums-collapse matmul (covering the DVE accumulator
                latency); post[jb] fillers run after the normalize."""
                fi = iter(fillers)

                def fill(n):
                    for _ in range(n):
                        it = next(fi, None)
                        if it is None:
                            return
                        it()

                def s_step(jb, ks):
                    m = ks - 4 * jb
                    q0 = m * P if m > 0 else 0
                    pst = pstp.tile([P, 512], f32, tag="pst")
                    nc.tensor.matmul(
                        pst[:, q0:],
                        kt[:, ks * P:(ks + 1) * P],
                        qt[:, jb * 512 + q0:(jb + 1) * 512],
                        start=True, stop=True)
                    pt = ptp.tile([P, 512], bf, tag="pt")
                    nc.scalar.activation(pt[:, q0:], pst[:, q0:], EXP,
                                         scale=SCALE)
                    return pt, q0, m

                for jb in range(NTB):
                    nk = 4 * (jb + 1)
                    po = pop.tile([P, 512], f32, tag="po")
                    acc = accp.tile([P, 512], f32r, tag="acc")
                    # depth-2 pipeline: the S matmul (and exp) for chunk
                    # ks+2 issues before the AV matmul of chunk ks, so
                    # AV never waits on the Activation engine.
                    pending = [s_step(jb, 0)]
                    if nk > 1:
                        pending.append(s_step(jb, 1))
                    with nc.allow_low_precision("f32r softmax sums ok"):
                        for ks in range(nk):
                            if ks + 2 < nk:
                                pending.append(s_step(jb, ks + 2))
                            pt, q0, m = pending.pop(0)
                            if m >= 0:
                                nc.vector.tensor_tensor(
                                    pt[:, m * P:(m + 1) * P],
                                    pt[:, m * P:(m + 1) * P], tri_t, MULT)
                            if ks == 0:
                                nc.vector.tensor_copy(out=acc, in_=pt)
                            else:
                                nc.vector.tensor_tensor(
                                    acc[:, q0:], acc[:, q0:], pt[:, q0:],
                                    ADD)
                            nc.tensor.matmul(
                                po[:, q0:], v[:, ks], pt[:, q0:],
                                start=(ks == 0), stop=(ks == nk - 1))
                    fill(pre[jb])
                    # collapse partitions + broadcast sums in one matmul
                    pss = pssp.tile([P, 512], f32, tag="pss")
                    nc.tensor.matmul(pss, ones_t, acc,
                                     start=True, stop=True)
                    rec = recp.tile([P, 512], f32, tag="rec")
                    nc.vector.reciprocal(rec, pss)
                    nc.vector.tensor_tensor(
                        ot[h][:, jb * 512:(jb + 1) * 512], po, rec, MULT)
                    fill(post[jb])
                for it in fi:
                    it()

            def c_group(tch, ob, wob):
                def run():
                    ps = pap.tile([P, 512], f32, tag="pa")
                    for hh in range(HG):
                        nc.tensor.matmul(
                            ps, ot[hh][:, tch * P:(tch + 1) * P],
                            wob[:, hh],
                            start=(hh == 0), stop=(hh == HG - 1))
                    st = cstp.tile([P, 512], f32, tag="cst")
                    nc.scalar.copy(out=st, in_=ps)
                    nc.sync.dma_start(
                        out_d[tch * P:(tch + 1) * P,
                              ob * 512:(ob + 1) * 512], st)
                return run

            # ---------------- main schedule ----------------
            xts = dma_xt()
            w_cur = dma_w(0)
            w_nxt = dma_w(1)
            for _rep in range(repeat):
                items0, qkv0 = make_qkv_items(0, w_cur, xts)
                for it in items0:
                    it()
                qkv_next = None
                wob0 = None
                wob1 = None
                for h in range(HG):
                    if h + 2 < HG:
                        w_after = dma_w(h + 2)
                    if h == HG - 2:
                        wob0 = dma_wob(0)
                    if h == HG - 1:
                        wob1 = dma_wob(1)
                    if h + 1 < HG:
                        fillers, qkv_next = make_qkv_items(
                            h + 1, w_nxt, xts)
                        w_nxt = w_after if h + 2 < HG else None
                        pre, post = [1, 1, 1, 1], [0, 1, 2, 99]
                    else:
                        # first output-projection groups (ob=0) as
                        # fillers.  Group tch reads ot[7] q-block
                        # tch//4, so it may only run after that
                        # block's normalize: with this pre/post
                        # schedule group tch_k runs after q-block
                        # min(k, 3) while needing only block k//4.
                        fillers = [c_group(tch, 0, wob0)
                                   for tch in range(12)]
                        pre, post = [0, 1, 1, 1], [1, 2, 3, 99]
                    qt, kt = qkv0 if h == 0 else qkv_cur
                    attn(h, qt, kt, fillers, pre, post)
                    qkv_cur = qkv_next
                # ---- output projection (rest) + next-rep prefetch ----
                xts = dma_xt()
                w_cur = dma_w(0)
                w_nxt = dma_w(1)
                wob = {0: wob0, 1: wob1}
                for ob in range(4):
                    if ob + 2 <= 3:
                        wob[ob + 2] = dma_wob(ob + 2)
                    for tch in range(NTC):
                        if ob == 0 and tch < 12:
                            continue  # emitted as attn_7 fillers
                        c_group(tch, ob, wob[ob])()

    nc.finalize()
    return nc


def _get_nc():
    global _CACHED
    if _CACHED is None:
        _CACHED = _build()
    return _CACHED


def _prep_inputs(x, W_qkv, W_out, vmode=None):
    """Host-side shard + layout prep (bf16). Returns per-core maps."""
    import ml_dtypes
    bf = ml_dtypes.bfloat16
    f32 = np.float32
    x = np.asarray(x, dtype=f32)
    W_qkv = np.asarray(W_qkv, dtype=f32)
    W_out = np.asarray(W_out, dtype=f32)

    k_idx = np.arange(P)
    q_idx = np.arange(P)
    tri = (q_idx[None, :] >= k_idx[:, None]).astype(bf)   # [128,128]
    ones = np.ones((P, P), dtype=f32)

    per_g = {}
    for g in range(2):
        sl = slice(g * HG * D, (g + 1) * HG * D)
        # [h, p, r, cs, d]: element = W_r[h*128+d, cs*128+p], r in {q,k}
        w2 = np.stack([
            W_qkv[r * C:(r + 1) * C][sl].reshape(HG, D, NCS, P)
            for r in range(2)
        ])  # [2, h, d, cs, p]
        w_a = np.ascontiguousarray(
            w2.transpose(1, 4, 0, 3, 2)).astype(bf)   # [h, p, 2, cs, d]
        # v weights per head-pair: [hp, p, cs, j] = Wv[hp*256+j, cs*128+p]
        wv = W_qkv[2 * C:3 * C][sl]
        wv2_a = np.ascontiguousarray(
            wv.reshape(HG // 2, 2 * D, NCS, P).transpose(0, 3, 2, 1)
        ).astype(bf)                                   # [4, p, cs, 256]
        wo_a = np.ascontiguousarray(
            W_out[:, sl].T.reshape(HG, P, C).transpose(1, 0, 2)
        ).astype(bf)                                   # [p, h, o]
        per_g[g] = (w_a, wv2_a, wo_a)

    in_maps = []
    for core in range(8):
        b, g = divmod(core, 2)
        xt = np.ascontiguousarray(x[b].T).reshape(NCS, P, T).astype(bf)
        w_a, wv2_a, wo_a = per_g[g]
        in_maps.append({
            "xt": xt, "w": w_a, "wv2": wv2_a, "wo": wo_a,
            "tri": tri, "ones": ones,
        })
    return in_maps


def kernel(x, W_qkv, W_out, *, trace=False, trace_cores=None):
    from concourse.bass_utils import run_bass_kernel_spmd

    nc = _get_nc()
    in_maps = _prep_inputs(x, W_qkv, W_out)
    r = run_bass_kernel_spmd(
        nc, in_maps, core_ids=list(range(8)),
        trace=trace, trace_cores=trace_cores)

    out = np.empty((B, T, C), dtype=np.float32)
    for b in range(B):
        out[b] = r.results[2 * b]["out"] + r.results[2 * b + 1]["out"]
    if trace:
        kernel.last_results = r
    return out


# revision 3
# speedup vs baseline: 2.1710x; 2.1368x over previous
"""Causal self-attention Trainium2 kernel (8 NeuronCores, SPMD) — v2.

Problem (hardcoded): x [4, 2048, 2048] f32, W_qkv [6144, 2048], W_out
[2048, 2048], 16 heads x 128 dim, causal softmax attention + output
projection.

Sharding: core c = 2*b + g handles batch b (4) and head-group g (2 groups
of 8 heads).  Host sums the two partial output projections per batch.

v2 design (PE-cycle minimization; the kernel is tensor-engine bound):
  * all matmul operands in bf16 (PSUM accumulation stays f32) — bf16 runs
    1 cycle/row at any free width, so causal masking can shrink tiles to
    128 columns (f32r needs >=256-wide tiles for full rate).
  * softmax denominators accumulated on the Vector engine (f32 running
    sum of the exp tiles) instead of per-chunk ones-matmuls on the PE;
    a single ones[128,128] matmul per q-block collapses partitions AND
    broadcasts the sums to all 128 partitions (replacing the separate
    reciprocal-broadcast matmul).
  * fused per-head pipeline: head h's attention interleaves (on the PE
    instruction stream) with head h+1's QKV projection segments, so the
    Activation engine's exp chain never stalls the PE.
  * Q^T/K^T/V stay in SBUF (no DRAM round-trip); x tiles and per-head
    weights are re-DMA'd every repeat (steady-state cost), with the x
    reload for the next repeat overlapped with the output projection.
"""

import math

import numpy as np

B = 4
T = 2048
C = 2048
H = 16          # total heads
HG = 8          # heads per core (tensor-parallel group)
D = 128         # head dim
P = 128         # partitions
NCS = C // P    # 16 contraction subtiles
NTC = T // P    # 16 T chunks of 128
NTB = T // 512  # 4 T blocks of 512
SCALE = 1.0 / math.sqrt(D)

_CACHED = None
VMODE = "v2"


def _build(phases="abc", repeat=1, vmode=None):
    import concourse.mybir as mybir
    from concourse import bacc
    from concourse.tile import TileContext

    f32 = mybir.dt.float32
    f32r = mybir.dt.float32r
    bf = mybir.dt.bfloat16
    EXP = mybir.ActivationFunctionType.Exp
    MULT = mybir.AluOpType.mult
    ADD = mybir.AluOpType.add

    nc = bacc.Bacc("TRN2", target_bir_lowering=False)

    xt_d = nc.dram_tensor("xt", [NCS, P, T], bf, kind="ExternalInput")
    # packed per-head q/k weights: [h, p, r, cs, d] = W_r[h*128+d, cs*128+p]
    w_d = nc.dram_tensor("w", [HG, P, 2, NCS, D], bf, kind="ExternalInput")
    # v weights per head-PAIR (256-wide moving dim keeps the PE weight
    # loads hidden): [hp, p, cs, j] = W_v[hp*256+j, cs*128+p]
    wv2_d = nc.dram_tensor("wv2", [HG // 2, P, NCS, 2 * D], bf,
                           kind="ExternalInput")
    # output projection: [p, h, o] = W_out[o, sl + h*128 + p]
    wo_d = nc.dram_tensor("wo", [P, HG, C], bf, kind="ExternalInput")
    tri_d = nc.dram_tensor("tri", [P, P], bf, kind="ExternalInput")
    ones_d = nc.dram_tensor("ones", [P, P], f32r, kind="ExternalInput")
    out_d = nc.dram_tensor("out", [T, C], f32, kind="ExternalOutput")

    with TileContext(nc) as tc:
        with tc.tile_pool(name="persist", bufs=1) as persist, \
             tc.tile_pool(name="xt", bufs=1) as xtp, \
             tc.tile_pool(name="ot", bufs=1) as otp, \
             tc.tile_pool(name="wp", bufs=2) as wp, \
             tc.tile_pool(name="wvp", bufs=2) as wvp, \
             tc.tile_pool(name="wob", bufs=2) as wobp, \
             tc.tile_pool(name="qk", bufs=4) as qkp, \
             tc.tile_pool(name="vv", bufs=4) as vp, \
             tc.tile_pool(name="pt", bufs=5) as ptp, \
             tc.tile_pool(name="acc", bufs=2) as accp, \
             tc.tile_pool(name="rec", bufs=1) as recp, \
             tc.tile_pool(name="cst", bufs=2) as cstp, \
             tc.tile_pool(name="pa", bufs=3, space="PSUM") as pap, \
             tc.tile_pool(name="pst", bufs=2, space="PSUM") as pstp, \
             tc.tile_pool(name="po", bufs=2, space="PSUM") as pop, \
             tc.tile_pool(name="pss", bufs=1, space="PSUM") as pssp:

            tri_t = persist.tile([P, P], bf, tag="tri")
            nc.sync.dma_start(tri_t, tri_d[:])
            ones_t = persist.tile([P, P], f32r, tag="ones")
            nc.sync.dma_start(ones_t, ones_d[:])

            ot = [persist.tile([P, T], bf, tag=f"ot{h}", name=f"ot{h}")
                  for h in range(HG)]

            def dma_xt():
                xts = []
                for cs in range(NCS):
                    t_ = xtp.tile([P, T], bf, tag=f"xt{cs}")
                    nc.sync.dma_start(t_, xt_d[cs])
                    xts.append(t_)
                return xts

            def dma_w(h):
                w_t = wp.tile([P, 2, NCS, D], bf, tag="w")
                nc.sync.dma_start(w_t, w_d[h])
                if h % 2 == 0:
                    wv_t = wvp.tile([P, NCS, 2 * D], bf, tag="wv")
                    nc.sync.dma_start(wv_t, wv2_d[h // 2])
                    return (w_t, wv_t)
                return (w_t, None)

            def dma_wob(ob):
                t_ = wobp.tile([P, HG, 512], bf, tag="wob")
                nc.sync.dma_start(t_, wo_d[:, :, ob * 512:(ob + 1) * 512])
                return t_

            vtiles = {}

            def make_qkv_items(h, w_pair, xts):
                """Uniform PE segments (~8192 cycles each) computing
                Q^T, K^T (D-part x T) for head h, plus — on even h — V
                (k-part x D chunks) for the PAIR (h, h+1) with a
                256-wide moving dim so PE weight loads stay hidden.
                Returns (items, (qt, kt))."""
                w_t, wv_t = w_pair
                qt = qkp.tile([P, T], bf, tag="qt")
                kt = qkp.tile([P, T], bf, tag="kt")
                if h % 2 == 0:
                    vtiles[h] = vp.tile([P, NTC, D], bf, tag="v",
                                        name=f"v{h}")
                    vtiles[h + 1] = vp.tile([P, NTC, D], bf, tag="v",
                                            name=f"v{h + 1}")

                def qk_seg(r, tb, dst):
                    def run():
                        ps = pap.tile([P, 512], f32, tag="pa")
                        for cs in range(NCS):
                            nc.tensor.matmul(
                                ps, w_t[:, r, cs],
                                xts[cs][:, tb * 512:(tb + 1) * 512],
                                start=(cs == 0), stop=(cs == NCS - 1))
                        nc.scalar.copy(
                            out=dst[:, tb * 512:(tb + 1) * 512], in_=ps)
                    return run

                def v2_seg(jj):
                    va, vb = vtiles[h], vtiles[h + 1]

                    def run():
                        ps = pap.tile([P, 512], f32, tag="pa")
                        for i in range(2):
                            tch = jj * 2 + i
                            for cs in range(NCS):
                                nc.tensor.matmul(
                                    ps[:, i * 256:(i + 1) * 256],
                                    xts[cs][:, tch * P:(tch + 1) * P],
                                    wv_t[:, cs],
                                    start=(cs == 0), stop=(cs == NCS - 1))
                        pv = ps.rearrange("p (i h j) -> p i h j", i=2, h=2)
                        nc.scalar.copy(
                            out=va[:, jj * 2:(jj + 1) * 2, :],
                            in_=pv[:, :, 0, :])
                        nc.scalar.copy(
                            out=vb[:, jj * 2:(jj + 1) * 2, :],
                            in_=pv[:, :, 1, :])
                    return run

                items = [qk_seg(0, tb, qt) for tb in range(NTB)]
                items += [qk_seg(1, tb, kt) for tb in range(NTB)]
                if h % 2 == 0:
                    items += [v2_seg(jj) for jj in range(8)]
                return items, (qt, kt)

            def attn(h, qt, kt, fillers, pre, post):
                v = vtiles[h]
                """Attention for head h; fillers are PE work items
                (next head's QKV segs / first output-proj groups)
                interleaved so the PE never waits on the exp chain.
                pre[jb] fillers run between the last AV matmul and the
                sums-collapse matmul (covering the DVE accumulator
                latency); post[jb] fillers run after the normalize."""
                fi = iter(fillers)

                def fill(n):
                    for _ in range(n):
                        it = next(fi, None)
                        if it is None:
                            return
                        it()

                def s_step(jb, ks):
                    m = ks - 4 * jb
                    q0 = m * P if m > 0 else 0
                    pst = pstp.tile([P, 512], f32, tag="pst")
                    nc.tensor.matmul(
                        pst[:, q0:],
                        kt[:, ks * P:(ks + 1) * P],
                        qt[:, jb * 512 + q0:(jb + 1) * 512],
                        start=True, stop=True)
                    pt = ptp.tile([P, 512], bf, tag="pt")
                    nc.scalar.activation(pt[:, q0:], pst[:, q0:], EXP,
                                         scale=SCALE)
                    return pt, q0, m

                for jb in range(NTB):
                    nk = 4 * (jb + 1)
                    po = pop.tile([P, 512], f32, tag="po")
                    acc = accp.tile([P, 512], f32r, tag="acc")
                    # depth-2 pipeline: the S matmul (and exp) for chunk
                    # ks+2 issues before the AV matmul of chunk ks, so
                    # AV never waits on the Activation engine.
                    pending = [s_step(jb, 0)]
                    with nc.allow_low_precision("f32r softmax sums ok"):
                        for ks in range(nk):
                            if ks + 1 < nk:
                                pending.append(s_step(jb, ks + 1))
                            pt, q0, m = pending.pop(0)
                            if m >= 0:
                                nc.vector.tensor_tensor(
                                    pt[:, m * P:(m + 1) * P],
                                    pt[:, m * P:(m + 1) * P], tri_t, MULT)
                            if ks == 0:
                                nc.vector.tensor_copy(out=acc, in_=pt)
                            else:
                                nc.vector.tensor_tensor(
                                    acc[:, q0:], acc[:, q0:], pt[:, q0:],
                                    ADD)
                            nc.tensor.matmul(
                                po[:, q0:], v[:, ks], pt[:, q0:],
                                start=(ks == 0), stop=(ks == nk - 1))
                    fill(pre[jb])
                    # collapse partitions + broadcast sums in one matmul
                    pss = pssp.tile([P, 512], f32, tag="pss")
                    nc.tensor.matmul(pss, ones_t, acc,
                                     start=True, stop=True)
                    rec = recp.tile([P, 512], f32, tag="rec")
                    nc.vector.reciprocal(rec, pss)
                    nc.vector.tensor_tensor(
                        ot[h][:, jb * 512:(jb + 1) * 512], po, rec, MULT)
                    fill(post[jb])
                for it in fi:
                    it()

            def c_group(tch, ob, wob):
                def run():
                    ps = pap.tile([P, 512], f32, tag="pa")
                    for hh in range(HG):
                        nc.tensor.matmul(
                            ps, ot[hh][:, tch * P:(tch + 1) * P],
                            wob[:, hh],
                            start=(hh == 0), stop=(hh == HG - 1))
                    st = cstp.tile([P, 512], f32, tag="cst")
                    nc.scalar.copy(out=st, in_=ps)
                    nc.sync.dma_start(
                        out_d[tch * P:(tch + 1) * P,
                              ob * 512:(ob + 1) * 512], st)
                return run

            # ---------------- main schedule ----------------
            xts = dma_xt()
            w_cur = dma_w(0)
            w_nxt = dma_w(1)
            for _rep in range(repeat):
                items0, qkv0 = make_qkv_items(0, w_cur, xts)
                for it in items0:
                    it()
                qkv_next = None
                wob0 = None
                wob1 = None
                for h in range(HG):
                    if h + 2 < HG:
                        w_after = dma_w(h + 2)
                    if h == HG - 2:
                        wob0 = dma_wob(0)
                    if h == HG - 1:
                        wob1 = dma_wob(1)
                    if h + 1 < HG:
                        fillers, qkv_next = make_qkv_items(
                            h + 1, w_nxt, xts)
                        w_nxt = w_after if h + 2 < HG else None
                        pre, post = [1, 1, 1, 1], [0, 1, 2, 99]
                    else:
                        # first output-projection groups (ob=0) as
                        # fillers.  Group tch reads ot[7] q-block
                        # tch//4, so it may only run after that
                        # block's normalize: with this pre/post
                        # schedule group tch_k runs after q-block
                        # min(k, 3) while needing only block k//4.
                        fillers = [c_group(tch, 0, wob0)
                                   for tch in range(12)]
                        pre, post = [0, 1, 1, 1], [1, 2, 3, 99]
                    qt, kt = qkv0 if h == 0 else qkv_cur
                    attn(h, qt, kt, fillers, pre, post)
                    qkv_cur = qkv_next
                # ---- output projection (rest) + next-rep prefetch ----
                xts = dma_xt()
                w_cur = dma_w(0)
                w_nxt = dma_w(1)
                wob = {0: wob0, 1: wob1}
                for ob in range(4):
                    if ob + 2 <= 3:
                        wob[ob + 2] = dma_wob(ob + 2)
                    for tch in range(NTC):
                        if ob == 0 and tch < 12:
                            continue  # emitted as attn_7 fillers
                        c_group(tch, ob, wob[ob])()

    nc.finalize()
    return nc


def _get_nc():
    global _CACHED
    if _CACHED is None:
        _CACHED = _build()
    return _CACHED


def _prep_inputs(x, W_qkv, W_out, vmode=None):
    """Host-side shard + layout prep (bf16). Returns per-core maps."""
    import ml_dtypes
    bf = ml_dtypes.bfloat16
    f32 = np.float32
    x = np.asarray(x, dtype=f32)
    W_qkv = np.asarray(W_qkv, dtype=f32)
    W_out = np.asarray(W_out, dtype=f32)

    k_idx = np.arange(P)
    q_idx = np.arange(P)
    tri = (q_idx[None, :] >= k_idx[:, None]).astype(bf)   # [128,128]
    ones = np.ones((P, P), dtype=f32)

    per_g = {}
    for g in range(2):
        sl = slice(g * HG * D, (g + 1) * HG * D)
        # [h, p, r, cs, d]: element = W_r[h*128+d, cs*128+p], r in {q,k}
        w2 = np.stack([
            W_qkv[r * C:(r + 1) * C][sl].reshape(HG, D, NCS, P)
            for r in range(2)
        ])  # [2, h, d, cs, p]
        w_a = np.ascontiguousarray(
            w2.transpose(1, 4, 0, 3, 2)).astype(bf)   # [h, p, 2, cs, d]
        # v weights per head-pair: [hp, p, cs, j] = Wv[hp*256+j, cs*128+p]
        wv = W_qkv[2 * C:3 * C][sl]
        wv2_a = np.ascontiguousarray(
            wv.reshape(HG // 2, 2 * D, NCS, P).transpose(0, 3, 2, 1)
        ).astype(bf)                                   # [4, p, cs, 256]
        wo_a = np.ascontiguousarray(
            W_out[:, sl].T.reshape(HG, P, C).transpose(1, 0, 2)
        ).astype(bf)                                   # [p, h, o]
        per_g[g] = (w_a, wv2_a, wo_a)

    in_maps = []
    for core in range(8):
        b, g = divmod(core, 2)
        xt = np.ascontiguousarray(x[b].T).reshape(NCS, P, T).astype(bf)
        w_a, wv2_a, wo_a = per_g[g]
        in_maps.append({
            "xt": xt, "w": w_a, "wv2": wv2_a, "wo": wo_a,
            "tri": tri, "ones": ones,
        })
    return in_maps


def kernel(x, W_qkv, W_out, *, trace=False, trace_cores=None):
    from concourse.bass_utils import run_bass_kernel_spmd

    nc = _get_nc()
    in_maps = _prep_inputs(x, W_qkv, W_out)
    r = run_bass_kernel_spmd(
        nc, in_maps, core_ids=list(range(8)),
        trace=trace, trace_cores=trace_cores)

    out = np.empty((B, T, C), dtype=np.float32)
    for b in range(B):
        out[b] = r.results[2 * b]["out"] + r.results[2 * b + 1]["out"]
    if trace:
        kernel.last_results = r
    return out
